# revision 12
# baseline (speedup 1.0000x reference)
"""Multi-head attention Trainium2 kernel (8 NeuronCores).

Problem: B=4, N=2048, D=64, H=12 multi-head attention with per-head QKV
projections, softmax attention, concat + output projection (fp32).

Sharding: 8 cores = 4 batches x 2 head-groups (6 heads each; the
"tensor parallel over heads" option from the sharding hint). Each core
emits per-head UNNORMALIZED output projections [Wo_h^T OT_h ; denom_h]
for its batch; the host applies the softmax normalization (a per-query
divide that commutes with the output projection), sums the head/group
partials (the reduce after the output projection), transposes, and adds
the output bias. Pushing the normalize to the host removes the on-device
reciprocal-broadcast + multiply + accumulate chain entirely.

Device algorithm (per core; fp32 data; matmuls float32r = full-rate
single-pass fp32; P/V in bf16). ScalarE (the exp stream over 6 x 2048^2
scores = 188us busy at 1 elem/cycle/partition) and the PE (~175us of
matmul columns) are near-balanced bottlenecks, so the schedule keeps
ScalarE 100% fed while the PE fills its per-tile slack with AV and
projection work:

  - x arrives host-pre-transposed as xT [64, 2048], augmented on-device
    with a ones partition-row; all projection weight stacks carry their
    bias as a 65th contraction row, so projections emit x@W+b directly
    and the PSUM->SBUF moves are plain copies (no bias DMAs, no adds)
  - Q/K projections are PAIR-PACKED single matmuls: lhsT [65, 128]
    blocks [Wq_even | Wk_odd] and [Wk_even | Wq_odd] produce
    128-partition PSUM outputs whose halves move base-aligned to
    QT2/KT2 [128, 2048] (even head rows 0:64, odd rows 64:128)
  - V natural [k, e] for all 6 heads at once, stored interleaved as
    [V_h | 1] (65-wide groups); the ones column makes the AV matmul emit
    [OT ; softmax denominator]
  - scores transposed ST[k, q] = K @ Q.T via row-packed matmul pairs
    (tile_position row groups 0/64), grouped 3 x 512 q-slots per 3-bank
    PSUM tile; exp on ScalarE straight out of PSUM with the 1/sqrt(D)
    scale fused (no max-subtraction needed: |scores| <~ 6 in fp32)
  - FINE-GRAINED INTERLEAVE: the previous iteration's 32 AV matmuls +
    finalize items + the next pair's projections are drained ~4 items
    after each score tile, so ScalarE never waits more than one tile and
    the PE never idles at iteration boundaries. PSUM: pscore 2 x 3 banks
    double-buffered scores, one persistent AV-accumulator bank, one
    persistent scratch bank for proj/outproj outputs.
  - The final iteration's AV runs during its own score tiles (second
    accumulator in a retiring pscore bank; ScalarE handles the last
    PSUM->SBUF copies once its exps are done) to compress the drain tail.

The walrus build here accepts only one sync-wait per instruction, so a
BIR post-pass splits Tile's multi-wait instructions onto NoOps (see
_split_excess_waits). Cost-model sim: ~197us (previous: 240us).
"""
import os
import sys

sys.path.insert(0, "/opt/trn_rl_repo")

# The kernel needs jax's axon (NeuronCore) backend. If the environment
# pinned JAX_PLATFORMS to something that excludes it (e.g. "cpu" for
# running the reference) and jax hasn't been imported yet, undo that.
_jp = os.environ.get("JAX_PLATFORMS")
if _jp and "axon" not in _jp and "jax" not in sys.modules:
    os.environ["JAX_PLATFORMS"] = ""

import numpy as np

import concourse.bass as bass
import concourse.tile as tile
from concourse import mybir

B, N, D, H = 4, 2048, 64, 12
NH = 6            # heads per core
NPAIR = 3         # head pairs per core
NKC = N // 128    # 16 k-chunks
QW = 512          # q tile width
NQC = N // QW     # 4 q-chunks
SLOTS = 2 * NKC   # 32 matmul outputs of QW cols per iteration
NTILE = (SLOTS + 2) // 3  # 11 score tiles per iteration (10x3 + 1x2)
NIT = NPAIR * NQC
F32 = mybir.dt.float32
F32R = mybir.dt.float32r
BF16 = mybir.dt.bfloat16

# ---------------------------------------------------------------------------
# This walrus build accepts only ONE sync wait command per instruction
# ("Too many sync wait commands" codegen error otherwise), while Tile emits
# instructions with several semaphore waits. Split excess waits onto NoOp
# instructions inserted just before the offender (same engine, so engine
# program order makes them execute first) by rewriting the BIR JSON on its
# way into the backend compiler.
# ---------------------------------------------------------------------------
_MAXW = 1


def _split_excess_waits(bir: dict) -> dict:
    counter = [0]

    def fix_block(b):
        insts = b.get("instructions")
        if insts:
            out = []
            for ins in insts:
                si = ins.get("sync_info")
                w = (si or {}).get("on_wait") or []
                if len(w) > _MAXW:
                    for k in range(0, len(w) - _MAXW, _MAXW):
                        counter[0] += 1
                        out.append({
                            "name": f"WSPL-{counter[0]}",
                            "opcode": "NoOp",
                            "engine": ins["engine"],
                            "ins": [],
                            "outs": [],
                            "debug": ins.get("debug", 0),
                            "sync_info": {
                                "on_wait": w[k:k + _MAXW],
                                "on_update": [],
                            },
                        })
                    si["on_wait"] = w[len(w) - _MAXW:]
                out.append(ins)
            b["instructions"] = out
        for sb in b.get("blocks", []) or []:
            fix_block(sb)

    for fn in bir.get("functions", []):
        for blk in fn.get("blocks", []):
            fix_block(blk)
    return bir


def _install_wait_split_hook():
    import json as _json

    import concourse.bass2jax as _b2j
    import concourse.bass_utils as _bu

    if getattr(_bu, "_wait_split_installed", False):
        return
    _orig = _bu.compile_bir_kernel

    def _cbk(bir_json, tmpdir, neff_name="file.neff"):
        if isinstance(bir_json, str):
            bir_json = bir_json.encode()
        d = _json.loads(bir_json)
        d = _split_excess_waits(d)
        return _orig(_json.dumps(d).encode(), tmpdir, neff_name=neff_name)

    _bu.compile_bir_kernel = _cbk
    _b2j.compile_bir_kernel = _cbk
    _bu._wait_split_installed = True


_install_wait_split_hook()


def build_nc(reps=1):
    nc = bass.Bass("TRN2", target_bir_lowering=False, debug=False)

    xt_d = nc.dram_tensor("xt", [D, N], F32R, kind="ExternalInput")
    # pair-packed Q/K blocks with bias row 64: per pair p, cols
    # [256p:256p+128] = [Wq_{2p} | Wk_{2p+1}], next 128 = [Wk_{2p} | Wq_{2p+1}]
    wqk_d = nc.dram_tensor("wqk", [D + 1, NPAIR * 256], F32R,
                           kind="ExternalInput")
    wv_d = nc.dram_tensor("wv", [D + 1, NH * D], F32R, kind="ExternalInput")
    wo_d = nc.dram_tensor("wo", [D, NH * D], F32R, kind="ExternalInput")
    # per-(pair,qc) blocks [65, 1024]: rows 0:64 = raw po for (even, odd)
    # head, row 64 = their softmax denominators; host divides and reduces
    po_d = nc.dram_tensor("po", [D + 1, NIT * 2 * QW], F32,
                          kind="ExternalOutput")

    with tile.TileContext(nc) as tc:
        with (
            tc.tile_pool(name="ptmp", bufs=3) as ptmp,
            tc.tile_pool(name="postg", bufs=2) as postg,
            tc.tile_pool(name="singles", bufs=1) as singles,
            tc.tile_pool(name="pP", bufs=22) as pP,
        ):
            xTa = singles.tile([D + 1, N], F32R)
            wqk_sb = singles.tile([D + 1, NPAIR * 256], F32R)
            wv_sb = singles.tile([D + 1, NH * D], F32R)
            wo_sb = singles.tile([D, NH * D], F32R)
            ones_f32 = singles.tile([128, NKC * NH], F32)
            QT2 = [singles.tile([128, N], F32R, name=f"QT2_{i}",
                                tag=f"QT2_{i}") for i in range(NPAIR)]
            KT2 = [singles.tile([128, N], F32R, name=f"KT2_{i}",
                                tag=f"KT2_{i}") for i in range(NPAIR)]
            Vn = singles.tile([128, NKC, NH, D + 1], BF16)

            # --- prologue DMAs, first-needed first (HWDGE serializes
            # transfer setup ~625ns each, so order matters) ---
            nc.sync.dma_start(xTa[0:D, 0:QW], xt_d[:, 0:QW])
            nc.sync.dma_start(wqk_sb[:], wqk_d[:])
            # ones row for the first q-chunk (bias contraction row)
            nc.vector.memset(xTa[D:D + 1, 0:QW].bitcast(F32), 1.0)

            # preload the exp table during the input DMAs so the first
            # real exp doesn't pay the ACT_TABLE_LOAD
            nc.vector.memset(ones_f32[:, 0:1], 0.0)
            nc.scalar.activation(
                ones_f32[:, 0:1], ones_f32[:, 0:1],
                mybir.ActivationFunctionType.Exp, scale=1.0,
            )

            # ones columns of the [V_h | 1] groups (fused softmax denom)
            nc.vector.memset(ones_f32[:], 1.0)
            nc.vector.tensor_copy(
                Vn[:, :, :, D:D + 1],
                ones_f32[:].rearrange("p (c h) -> p c h", c=NKC)[:, :, :, None],
            )

            for _rep in range(reps):
                with (
                    tc.tile_pool(name="pscore", bufs=2, space="PSUM") as pscore,
                    tc.tile_pool(name="pacc", bufs=1, space="PSUM") as pacc,
                    tc.tile_pool(name="pscr", bufs=1, space="PSUM") as pscr,
                ):
                    # persistent single-bank tiles: AV accumulator and
                    # proj/outproj scratch. Users are serialized through
                    # Tile WAR/RAW deps; the item queue spaces them so the
                    # PE never waits long.
                    pav = pacc.tile([128, QW], F32)
                    scr = pscr.tile([128, QW], F32)

                    def proj_block(p, qc, col, dst_lo, dst_hi, psum):
                        qs = slice(qc * QW, (qc + 1) * QW)
                        b = 2 * p + (0 if col % 2 == 0 else 1)
                        nc.tensor.matmul(
                            psum[:],
                            wqk_sb[:, col * 128:(col + 1) * 128],
                            xTa[:, qs],
                            start=True, stop=True,
                        )
                        nc.vector.tensor_copy(dst_lo[0:64, qs], psum[0:64, :])
                        nc.vector.tensor_copy(dst_hi[64:128, qs],
                                              psum[64:128, :])

                    def emit_proj(p, qc, b_psum=None):
                        # pair-packed Q/K projection for pair p, q-chunk qc:
                        # A block [Wq_even | Wk_odd], B block [Wk_even |
                        # Wq_odd]; bias row folded into the matmul
                        proj_block(p, qc, 2 * p, QT2[p], KT2[p], scr)
                        proj_block(p, qc, 2 * p + 1, KT2[p], QT2[p],
                                   b_psum if b_psum is not None else scr)

                    def emit_v(c):
                        # V natural (+bias row) for all heads, one
                        # matmul/chunk; uses the (idle in iteration 0) pav
                        nc.tensor.matmul(
                            pav[:, 0:NH * D],
                            xTa[:, c * 128:(c + 1) * 128],
                            wv_sb[:],
                            start=True, stop=True,
                        )
                        nc.vector.tensor_copy(
                            Vn[:, c, :, 0:D],
                            pav[:, 0:NH * D].rearrange("p (h e) -> p h e",
                                                       h=NH),
                        )

                    class ScoreEmitter:
                        """Row-packed scores matmuls + exp for one (p, qc),
                        three QW-slots per 3-bank psum tile."""

                        def __init__(self, p, qc):
                            self.p, self.qc = p, qc
                            self.qs = slice(qc * QW, (qc + 1) * QW)
                            self.ptiles = []

                        def emit_tile(self, t):
                            lo = 3 * t
                            hi = min(lo + 3, SLOTS)
                            width = (hi - lo) * QW
                            stile = pscore.tile([128, 1536], F32, tag="sc",
                                                name="sc")
                            ptile = pP.tile([128, 1536], BF16, tag="pexp",
                                            name="pexp")
                            self.ptiles.append(ptile)
                            for s in range(lo, hi):
                                c, hh = s // 2, s % 2
                                base = 0 if hh == 0 else 64
                                ks = slice(c * 128, (c + 1) * 128)
                                nc.tensor.matmul(
                                    stile[:, (s - lo) * QW:(s - lo + 1) * QW],
                                    KT2[self.p][base:base + 64, ks],
                                    QT2[self.p][base:base + 64, self.qs],
                                    start=True, stop=True,
                                    tile_position=(base, 0),
                                )
                            nc.scalar.activation(
                                ptile[:, 0:width], stile[:, 0:width],
                                mybir.ActivationFunctionType.Exp,
                                scale=1.0 / 8.0,
                            )

                    def av_queue(p, qc, ptiles, final=False, alt=None):
                        """Item list: AV + finalize for iteration (p, qc).
                        final: h1 accumulates into `alt` (a retiring pscore
                        tile) and PSUM->SBUF moves go on ScalarE, which has
                        no exps left to run."""
                        idx = p * NQC + qc

                        def pslice(c, hi):
                            s = 2 * c + hi
                            return ptiles[s // 3][
                                :, (s % 3) * QW:(s % 3 + 1) * QW]

                        items = []
                        state = {}
                        acc = {0: pav, 1: alt if final else pav}

                        def avmm(hi, c):
                            def f():
                                nc.tensor.matmul(
                                    acc[hi][0:D + 1, 0:QW],
                                    Vn[:, c, 2 * p + hi, :],
                                    pslice(c, hi),
                                    start=(c == 0), stop=(c == NKC - 1),
                                    skip_group_check=True,
                                )
                            return f

                        def copy(dst, src, on_act):
                            if on_act:
                                nc.scalar.copy(dst, src)
                            else:
                                nc.vector.tensor_copy(dst, src)

                        def fin1(hi, on_act=False):
                            # drain the accumulator: raw OT + denominator
                            def f():
                                if 'pg' not in state:
                                    state['pg'] = postg.tile([D + 1, 2 * QW],
                                                             F32, tag="pg",
                                                             name="pg")
                                pg = state['pg']
                                ot = ptmp.tile([D, QW], F32R, tag="ot")
                                a = acc[hi]
                                copy(ot[:], a[0:D, 0:QW], on_act)
                                copy(pg[D:D + 1, hi * QW:(hi + 1) * QW],
                                     a[D:D + 1, 0:QW], on_act)
                                state[hi] = ot
                            return f

                        def fin2(hi):
                            # output projection on the raw OT
                            def f():
                                nc.tensor.matmul(
                                    scr[0:D, :],
                                    wo_sb[:, (2 * p + hi) * D:
                                          (2 * p + hi + 1) * D],
                                    state[hi][:],
                                    start=True, stop=True,
                                )
                            return f

                        def fin3(hi, on_act=False):
                            # stage po + (second head) ship the block
                            def f():
                                pg = state['pg']
                                copy(pg[0:D, hi * QW:(hi + 1) * QW],
                                     scr[0:D, :], on_act)
                                if hi == 1:
                                    nc.sync.dma_start(
                                        po_d[:, idx * 2 * QW:
                                             (idx + 1) * 2 * QW],
                                        pg[:],
                                    )
                            return f

                        if not final:
                            for c in range(NKC):
                                items.append(avmm(0, c))
                            # position NKC: external proj items insert here —
                            # scr write order must stay psA,psB,po0,po1 so
                            # the in-order DVE never inverts a WAR dep
                            items.append(fin1(0))
                            items.append(fin2(0))
                            for c in range(NKC):
                                items.append(avmm(1, c))
                            items.append(fin3(0))
                            items.append(fin1(1))
                            items.append(fin2(1))
                            items.append(fin3(1))
                        else:
                            # both heads accumulate concurrently (separate
                            # banks), chunk-paired so each exp tile is
                            # consumed as soon as it lands
                            for c in range(NKC):
                                items.append(avmm(0, c))
                                items.append(avmm(1, c))
                            items.append(fin1(0))
                            items.append(fin2(0))
                            items.append(fin1(1, on_act=True))
                            items.append(fin3(0))
                            items.append(fin2(1))
                            items.append(fin3(1, on_act=True))
                        return items

                    # ---------------- iteration 0 (prologue) ----------------
                    se = ScoreEmitter(0, 0)

                    def dma_xt(qc):
                        def f():
                            nc.sync.dma_start(
                                xTa[0:D, qc * QW:(qc + 1) * QW],
                                xt_d[:, qc * QW:(qc + 1) * QW],
                            )
                            nc.vector.memset(
                                xTa[D:D + 1,
                                    qc * QW:(qc + 1) * QW].bitcast(F32), 1.0)
                        return f

                    it0 = {t: [] for t in range(NTILE)}
                    it0[0] += [dma_xt(1),
                               lambda: nc.sync.dma_start(wv_sb[:], wv_d[:])]
                    it0[1] += [dma_xt(2), lambda: emit_proj(0, 1)]
                    it0[2] += [lambda c=c: emit_v(c) for c in range(0, 3)]
                    it0[3] += [dma_xt(3),
                               lambda: nc.sync.dma_start(wo_sb[:], wo_d[:])]
                    it0[4] += [lambda: emit_proj(0, 2)]
                    it0[4] += [lambda c=c: emit_v(c) for c in range(3, 6)]
                    it0[5] += [lambda c=c: emit_v(c) for c in range(6, 8)]
                    it0[6] += [lambda: emit_proj(0, 3)]
                    it0[6] += [lambda c=c: emit_v(c) for c in range(8, 10)]
                    it0[7] += [lambda c=c: emit_v(c) for c in range(10, 12)]
                    it0[8] += [lambda c=c: emit_v(c) for c in range(12, 14)]
                    it0[9] += [lambda c=c: emit_v(c) for c in range(14, 16)]

                    # pair-0 qc-0 projection: B block into the free pav bank
                    # so it doesn't serialize behind A's PSUM drain
                    emit_proj(0, 0, b_psum=pav)
                    for t in range(NTILE):
                        se.emit_tile(t)
                        for f in it0[t]:
                            f()
                    prev = (0, 0, se.ptiles)

                    # ---------------- steady state ----------------
                    for it in range(1, NIT + 1):
                        final = it == NIT
                        if final:
                            # retiring pscore tile as the second AV
                            # accumulator for the compressed tail
                            alt = pscore.tile([128, 1536], F32, tag="sc",
                                              name="altacc")
                            queue = av_queue(*prev, final=True, alt=alt)
                            for f in queue:
                                f()
                            continue
                        queue = av_queue(*prev)
                        p, qc = divmod(it, NQC)
                        if p + 1 < NPAIR and qc >= 1:
                            queue.insert(NKC, lambda p=p, qc=qc:
                                         emit_proj(p + 1, qc - 1))
                            if qc == NQC - 1:
                                queue.insert(NKC + 1, lambda p=p:
                                             emit_proj(p + 1, NQC - 1))
                        se = ScoreEmitter(p, qc)
                        per = (len(queue) + NTILE - 1) // NTILE
                        qi = 0
                        for t in range(NTILE):
                            se.emit_tile(t)
                            take = min(per, len(queue) - qi)
                            for f in queue[qi:qi + take]:
                                f()
                            qi += take
                        for f in queue[qi:]:
                            f()
                        prev = (p, qc, se.ptiles)

    return nc


_NC_CACHE = {}


def _get_nc(reps=1):
    if reps not in _NC_CACHE:
        _NC_CACHE[reps] = build_nc(reps)
    return _NC_CACHE[reps]


def prep_in_maps(x, Wq, Wk, Wv, bq, bk, bv, Wo, bo):
    x = np.asarray(x, dtype=np.float32)
    Wq = np.asarray(Wq, dtype=np.float32)
    Wk = np.asarray(Wk, dtype=np.float32)
    Wv = np.asarray(Wv, dtype=np.float32)
    bq = np.asarray(bq, dtype=np.float32)
    bk = np.asarray(bk, dtype=np.float32)
    bv = np.asarray(bv, dtype=np.float32)
    Wo = np.asarray(Wo, dtype=np.float32)

    in_maps = []
    for core in range(8):
        g = core % 2
        b = core // 2
        hs = slice(g * NH, (g + 1) * NH)
        wqg, wkg, wvg = Wq[hs], Wk[hs], Wv[hs]   # [NH, D, D]
        bqg, bkg, bvg = bq[hs], bk[hs], bv[hs]   # [NH, D]

        wqk = np.empty((D + 1, NPAIR * 256), dtype=np.float32)
        for p in range(NPAIR):
            wqk[0:D, 256 * p:256 * p + 64] = wqg[2 * p]
            wqk[0:D, 256 * p + 64:256 * p + 128] = wkg[2 * p + 1]
            wqk[0:D, 256 * p + 128:256 * p + 192] = wkg[2 * p]
            wqk[0:D, 256 * p + 192:256 * p + 256] = wqg[2 * p + 1]
            wqk[D, 256 * p:256 * p + 64] = bqg[2 * p]
            wqk[D, 256 * p + 64:256 * p + 128] = bkg[2 * p + 1]
            wqk[D, 256 * p + 128:256 * p + 192] = bkg[2 * p]
            wqk[D, 256 * p + 192:256 * p + 256] = bqg[2 * p + 1]

        wv = np.empty((D + 1, NH * D), dtype=np.float32)
        wv[0:D] = wvg.transpose(1, 0, 2).reshape(D, NH * D)
        wv[D] = bvg.reshape(NH * D)
        wo = np.ascontiguousarray(
            Wo[g * NH * D:(g + 1) * NH * D].reshape(NH, D, D)
            .transpose(1, 0, 2).reshape(D, NH * D)
        )
        in_maps.append({
            "xt": np.ascontiguousarray(x[b].T),
            "wqk": wqk, "wv": wv, "wo": wo,
        })
    return in_maps


def kernel(x, Wq, Wk, Wv, bq, bk, bv, Wo, bo, _trace=False, _reps=1):
    from concourse.bass_utils import run_bass_kernel_spmd

    bo = np.asarray(bo, dtype=np.float32)
    nc = _get_nc(_reps)
    in_maps = prep_in_maps(x, Wq, Wk, Wv, bq, bk, bv, Wo, bo)

    res = run_bass_kernel_spmd(
        nc, in_maps, core_ids=list(range(8)), trace=_trace
    )

    out = np.zeros((B, D, N), dtype=np.float32)
    for core in range(8):
        po = res.results[core]["po"]          # [65, NIT*2*QW]
        blocks = po.reshape(D + 1, NIT, 2, QW)
        o = blocks[0:D]                        # [D, NIT, 2, QW]
        den = blocks[D]                        # [NIT, 2, QW]
        contrib = (o / den[None]).sum(axis=2)  # [D, NIT, QW]
        # NIT blocks are (pair-major, qc-minor): sum pairs per qc
        contrib = contrib.reshape(D, NPAIR, NQC * QW).sum(axis=1)
        out[core // 2] += contrib
    out = np.ascontiguousarray(out.transpose(0, 2, 1)) + bo[None, None, :]

    if _trace:
        return out, res
    return out
